# revision 53
# baseline (speedup 1.0000x reference)
"""AdvancedCrossStreamAttention Trainium2 kernel (8-core SPMD, batch-sharded).

Reference computation (per batch b, stream pair i in {0,1,2}):
    q = Wq @ x_i + bq            [32, N]     N = T*J = 1600
    k = Wk @ x_{i+1} + bk        [32, N]
    v = Wv @ x_{i+1} + bv        [256, N]
    energy = q^T k               [N, N]
    attn = softmax(energy, -1)
    cross_i = v @ attn^T         [256, N]
    out = mean_i(cross_i * fw[i]) -> [B, C, T, J]

Sharding: batch 16 -> 2 per core across 8 cores; weights replicated.

v3 design notes (~224-226us measured, vs v2 at 247.9us, vs v1 at ~296us;
one run flaked to 270us — device-state variance, not schedule-dependent):
  - Energy matmuls in 4-way tile_position bursts (rows 0/32/64/96, two
    [128, 2, 512] psum pair-tiles = 4 banks): the PE overlaps ~2 streams,
    and the coarser quad pieces pace cleanly into the cross stream.
  - Exp tiles are published progressively per 512-col slice (a0: n cols
    0-512, a1: 512-1024, bt: 1024-1600) so the consumer's cross chunks 0-3
    start as soon as the first 7 exps + 13 vT tiles exist.
  - fill_work pulls AT MOST ONE energy piece per cross-chunk boundary: its
    psum WAR (the exp one quad back) is then already satisfied, so the 4
    matmuls dispatch dep-free.  Stalled matmuls otherwise sit in the PE
    wait-queue (depth 4) and split the cross stream mid-chunk; every such
    re-entry cost ~107ns on the next cross matmul (measured p90 216ns vs
    p50 110ns before this fix).
  - The lone 64-row m-chunk (1600 = 12*128 + 64) is ZERO-PADDED to 128
    rows (gpsimd memsets on the exp tiles + vt tail): K=64 cross matmuls
    stream at HALF rate on the PE (measured 213ns vs 107ns for 257 cols).
  - Input loads are host-rearranged to [128, 2*N] per (batch, stream) so
    each loads with ONE dma_start (each start costs ~600ns sequencer issue;
    v2 burned ~13us issuing 27 of them); weights are concatenated into 4
    combined tensors (wqk/bqk/wvta/bvba).  The 16 DMA engines round-robin
    across ACTIVE queues, so the critical s0/s1 are split half-and-half
    across the two fast hwdge rings (sync + ACT) and the bulk s2 load is
    DEPENDENCY-GATED behind a gpsimd read of q4 (which needs s0) so it
    cannot steal engine share during the ramp.  Batch-1 loads stay off the
    critical rings.  (Queueing ALL bulk loads behind one ring measured
    267us; gating b1 onto the gpsimd ring measured 228us — both worse.)
  - exp consumes a pair tile in one ACTIVATE ([128, 2, w] strided APs).
    ACT floor ~140us/core (elems at ~1.4GHz/partition + ~250ns/instr);
    PE floor ~209us busy -> PE is the binding engine at 88% of span.
  - Normalize is fused: acc = (cps * rinv) + acc via scalar_tensor_tensor
    (one DVE pass instead of mul+add), accumulating in bf16.
  - Output transposes moved off the device entirely: acc [n, c] bf16 is
    DMA-stored to y[b, n, c] (contiguous 512B rows) and the final [B,C,T,J]
    layout transpose + fp32 upcast happen in the host-side unshard step
    (adds ~2e-3 rel err from bf16 output, tolerance 2e-2).
  - The whole kernel is software-pipelined at emission level: unit u's cross
    matmul stream interleaves (generator "pieces") with unit u+1's
    projections/vT/energy+exp, fills only at cross-chunk boundaries.
  - Dead end (measured): fp8 DoubleRow cross would halve PE time but fails
    accuracy — e5m2 exp weights give 7.6e-2 rel err, e4m3 vT alone 2.4e-2
    (softmax here is spiky: sigma(energy)=3.6 -> v-quant noise transfers
    ~1:1 to the output).  Numpy-simulated on the real seed-0 inputs.

Infra workarounds for this walrus build:
  - SplitDrainTileContext + legalize_waits: codegen accepts only ONE sync
    wait per instruction; extra waits are hoisted onto single-wait
    EventSemaphore instructions.
"""

import sys
from contextlib import ExitStack

for _p in ("/opt/trn_rl_repo", "/root/.axon_site/_ro/trn_rl_repo"):
    if _p not in sys.path:
        sys.path.insert(0, _p)

import numpy as np

import concourse.bass as bass
import concourse.tile as tile
from concourse import mybir
from concourse.bass_utils import run_bass_kernel_spmd
from concourse.vector_clock import VectorClock, ScopedClock
from concourse.tile_sem_assignment import N_PROCS

F32 = mybir.dt.float32
BF16 = mybir.dt.bfloat16
F16 = mybir.dt.float16

B, C, T, J = 16, 256, 64, 25
N = T * J                      # 1600
C8 = C // 8                    # 32
NCORES = 8
BPC = B // NCORES              # batches per core

# n (and m) chunks over the 1600 spatial positions: 12 x 128 + 64
CHUNKS = [(i * 128, min(128, N - i * 128)) for i in range((N + 127) // 128)]
NCH = len(CHUNKS)              # 13
NPAIR = NCH // 2               # 6 full m-chunk pairs; chunk 12 is lone (pm=64)
NPAD = 13 * 128                # k4 padded so the lone chunk is 128-wide


class SplitDrainTileContext(tile.TileContext):
    """Tile exit drain emitted as one single-wait drain per hardware proc."""

    def _drain_and_barrier(self, tick_clock, wait_clock):
        gc = tick_clock.global_clock
        for p in range(N_PROCS):
            if gc[p] > 0:
                d = self.nc.sync.drain()
                wait_clock.add_sem_waits(
                    d.ins,
                    ScopedClock(
                        {None: VectorClock(
                            [gc[i] if i == p else 0 for i in range(N_PROCS)]
                        )}
                    ),
                )
        self.nc.all_engine_barrier()
        assert self.sems is not None
        popped = self.nc._tile_sem_poison_stack.pop()
        assert popped is self._sem_poison
        self.nc.clear_and_free_semaphores(list(self.sems.allocated().values()))
        self.nc.all_engine_barrier()


def legalize_waits(nc: bass.Bass, max_waits: int = 1) -> int:
    """Split instructions carrying more than ``max_waits`` sync waits."""
    n_split = 0
    for f in nc.m.functions:
        for blk in f.blocks:
            out = []
            changed = False
            for inst in blk.instructions:
                si = inst.sync_info
                if si is not None and si.on_wait is not None and len(si.on_wait) > max_waits:
                    waits = list(si.on_wait)
                    extra, keep = waits[:-max_waits], waits[-max_waits:]
                    for w in extra:
                        n_split += 1
                        ev = mybir.InstEventSemaphore(
                            name=f"Wsplit-{n_split}", ins=[], outs=[]
                        )
                        ev.engine = inst.engine
                        ev.sync_info = mybir.SyncInfo(on_wait=[w], on_update=[])
                        nc.register_instruction(ev)
                        out.append(ev)
                    inst.sync_info = mybir.SyncInfo(
                        on_wait=keep, on_update=list(si.on_update)
                    )
                    changed = True
                out.append(inst)
            if changed:
                blk.instructions = out
    return n_split


def _chain_gens(a, b):
    def gen():
        if a is not None:
            yield from a
        if b is not None:
            yield from b
    return gen()


def build_program() -> bass.Bass:
    nc = bass.Bass()

    # Inputs are host-rearranged so each logical tensor loads with ONE
    # dma_start (each start costs ~600ns of sequencer issue time; the v2
    # prologue burned ~13us just issuing 27 of them).
    s_par = [
        nc.declare_dram_parameter(f"s{i}", [BPC, 128, 2 * N], F16, isOutput=False)
        for i in range(3)
    ]
    wqk = nc.declare_dram_parameter("wqk", [128, 2 * 256], F16, isOutput=False)
    bqk = nc.declare_dram_parameter("bqk", [128, 2], F32, isOutput=False)
    wvta = nc.declare_dram_parameter("wvta", [128, 2 * 774], F16, isOutput=False)
    bvba = nc.declare_dram_parameter("bvba", [128, 774], F16, isOutput=False)
    y = nc.declare_dram_parameter("y", [BPC, N, C], BF16, isOutput=True)

    with SplitDrainTileContext(nc) as tc, ExitStack() as ctx:
        singles = ctx.enter_context(tc.tile_pool(name="singles", bufs=1))
        xsp = ctx.enter_context(tc.tile_pool(name="xsp", bufs=6))
        qkp = ctx.enter_context(tc.tile_pool(name="qkp", bufs=4))
        vtp = ctx.enter_context(tc.tile_pool(name="vtp", bufs=2 * NCH))
        expa0p = ctx.enter_context(tc.tile_pool(name="expa0p", bufs=14))
        expa1p = ctx.enter_context(tc.tile_pool(name="expa1p", bufs=14))
        expbp = ctx.enter_context(tc.tile_pool(name="expbp", bufs=14))
        accp = ctx.enter_context(tc.tile_pool(name="accp", bufs=2 * NCH))
        smallp = ctx.enter_context(tc.tile_pool(name="smallp", bufs=4))
        # PSUM budget = 8 banks: shared (proj/vt) 2x1, cross 2x1, energy 2x2
        shared_ps = ctx.enter_context(tc.tile_pool(name="shared_ps", bufs=2, space="PSUM"))
        cps_ps = ctx.enter_context(tc.tile_pool(name="cps_ps", bufs=2, space="PSUM"))
        eng_ps = ctx.enter_context(tc.tile_pool(name="eng_ps", bufs=2, space="PSUM"))

        # --- batched loads: x tiles first (critical path), weights alongside
        # on the other ring.  One dma_start per logical tensor.
        loaded_x = [[None] * 3 for _ in range(BPC)]

        def load_x(b, s, eng):
            t = xsp.tile([128, 2, N], F16, tag="xs", name="xs")
            eng.dma_start(t[:], s_par[s][b])
            loaded_x[b][s] = t

        wqk_sb = singles.tile([128, 2, 256], F16, tag="wqk", name="wqk")
        bqk_sb = singles.tile([128, 2], F32, tag="bqk", name="bqk")
        wvta_sb = singles.tile([128, 2, 774], F16, tag="wvta", name="wvta")
        bvba_sb = singles.tile([128, 774], F16, tag="bvba", name="bvba")

        # Prologue DMA priority: unit 0 needs wqk + s0 + s1 first; batch-1
        # streams are deferred into unit 1's stage (needed ~60us later) so
        # they don't compete for HBM bandwidth during the ramp.
        nc.sync.dma_start(wqk_sb[:], wqk[:])
        nc.sync.dma_start(bqk_sb[:], bqk[:])
        # s0/s1 are the prologue's critical path: split each across BOTH
        # fast hwdge rings (sync + scalar) so queue-arbitration starvation
        # by the bulk s2/b1 loads can't stall either stream completely.
        t00 = xsp.tile([128, 2, N], F16, tag="xs", name="xs")
        t01 = xsp.tile([128, 2, N], F16, tag="xs", name="xs")
        loaded_x[0][0] = t00
        loaded_x[0][1] = t01
        nc.sync.dma_start(t00[:, 0, :], s_par[0][0, :, 0:N])
        nc.scalar.dma_start(t00[:, 1, :], s_par[0][0, :, N:2 * N])
        nc.scalar.dma_start(t01[:, 0, :], s_par[1][0, :, 0:N])
        nc.sync.dma_start(t01[:, 1, :], s_par[1][0, :, N:2 * N])
        nc.sync.dma_start(wvta_sb[:], wvta[:])
        nc.sync.dma_start(bvba_sb[:], bvba[:])
        # s2 is gated inside unit 0's generator (behind a gpsimd read of q4)
        # so its bulk transfer cannot steal DMA-engine share from s0/s1:
        # the 16 engines round-robin across ACTIVE queues, so a concurrent
        # bulk queue delays the critical loads by its fair share.

        # views into the combined weight tiles
        wq4t_sb = [wqk_sb[:, cc, 0:128] for cc in range(2)]
        wk4t_sb = [wqk_sb[:, cc, 128:256] for cc in range(2)]
        bq4_sb = bqk_sb[:, 0:1]
        wvt_sb = [[wvta_sb[:, cc, 258 * i:258 * i + 258] for cc in range(2)]
                  for i in range(3)]
        bvb_sb = [bvba_sb[:, 258 * i:258 * i + 257] for i in range(3)]

        # Warm-up during the input-DMA window: a dummy exp pulls the ~2.7us
        # ACT table load off the first real exp of the chain, and a burst of
        # dummy matmuls on the already-loaded projection weights ramps the
        # HAM clock-gate before real work starts.
        warm_exp = singles.tile([128, 1], BF16, tag="warm_exp", name="warm_exp")
        nc.scalar.activation(
            warm_exp[:], bqk_sb[:, 0:1], mybir.ActivationFunctionType.Exp
        )
        # 40 back-to-back warmup matmuls ~= 4.3us of sustained PE activity:
        # enough to flip the HAM clock-gate's 3.4us SHORT window to 8/8
        # (2.4GHz) BEFORE the real projections start.  14 matmuls (1.5us)
        # measured too short: the first qk projections ran at 1.2GHz
        # (p90 426ns vs 213 warm).  All of it hides in the ~7.7us window
        # while the s0/s1 input DMA is still streaming.
        warm_ps = shared_ps.tile([128, 512], F32, tag="sps", name="sps")
        for _ in range(40):
            nc.tensor.matmul(
                warm_ps[:128, :128],
                wq4t_sb[0],
                wk4t_sb[0],
                start=True,
                stop=True,
            )

        units = [(b, i) for b in range(BPC) for i in range(3)]
        NU = len(units)

        stage_out = {}       # u -> dict with progressive vt/a0/a1/bt lists
        batch_acc = {}       # b -> list of acc tiles

        def unit_stage_gen(u):
            """Emit unit u's projections, vT and energy+exp, yielding between
            pieces so the driver can interleave them into the previous unit's
            cross matmul stream.  Exp tiles are published progressively:
            a0 (n cols 0:512) -> cross chunks 0-3 can start, a1 (512:1024)
            -> chunks 4-7, bt (1024:1600) -> chunks 8-12."""
            b, i = units[u]
            st = {"vt": [], "a0": [None] * (NPAIR + 1), "a1": [None] * (NPAIR + 1),
                  "bt": [None] * (NPAIR + 1), "na0": 0, "na1": 0, "nbt": 0,
                  "done": False}
            stage_out[u] = st
            if u == 1:
                # deferred batch-1 input loads (issued early by the static
                # sequencer stream regardless, but off the critical rings)
                load_x(1, 0, nc.sync)
                load_x(1, 1, nc.gpsimd)
                load_x(1, 2, nc.sync)
            xq = loaded_x[b][i]
            xk = loaded_x[b][(i + 1) % 3]

            # --- q/k projections (4x-replicated rows via host-tiled weights)
            # (padding k4 to 1664 cols to make the lone m-chunk a full
            # 128-row energy chunk measured WORSE: 229us vs 225us)
            q4 = qkp.tile([128, N], F16, tag="q4", name="q4")
            k4 = qkp.tile([128, N], F16, tag="k4", name="k4")

            def qk_piece(dst, xsrc, wcol, with_bias, s0, w):
                ps = shared_ps.tile([128, 512], F32, tag="sps", name="sps")
                for cc in range(2):
                    nc.tensor.matmul(
                        ps[:, :w],
                        wqk_sb[:, cc, wcol:wcol + 128],
                        xsrc[:, cc, s0:s0 + w],
                        start=(cc == 0),
                        stop=(cc == 1),
                    )
                if not with_bias:
                    # k-bias is dropped exactly: softmax_m(q.(khat+bk)) ==
                    # softmax_m(q.khat) since q.bk is constant per row.
                    nc.vector.tensor_copy(dst[:, s0:s0 + w], ps[:, :w])
                else:
                    nc.vector.tensor_scalar_add(dst[:, s0:s0 + w], ps[:, :w], bqk_sb[:, 0:1])

            def q_piece(s0, w):
                qk_piece(q4, xq, 0, True, s0, w)

            def k_piece(s0, w):
                qk_piece(k4, xk, 128, False, s0, w)

            def emit_vt(mc):
                ms, pm = CHUNKS[mc]
                ps = shared_ps.tile([128, 512], F32, tag="sps", name="sps")
                for cc in range(2):
                    nc.tensor.matmul(
                        ps[:pm, :C + 2],
                        xk[:, cc, ms:ms + pm],
                        wvta_sb[:, cc, 258 * i:258 * i + 258],
                        start=(cc == 0),
                        stop=(cc == 1),
                    )
                vtile = vtp.tile([128, C + 1], F16, tag="vt", name="vt")
                nc.vector.tensor_add(vtile[:pm, :], ps[:pm, :C + 1], bvba_sb[:pm, 258 * i:258 * i + 257])
                if pm < 128:
                    # zero-pad so the lone m-chunk's cross matmuls run K=128
                    # (K=64 streams at half rate on the PE) without NaN risk
                    nc.gpsimd.memset(vtile[pm:128, :], 0.0)
                st["vt"].append(vtile)

            def emit_quad_A(qd, sl):
                """FOUR m-chunks' energy (cols sl:sl+512) in one burst: two
                [128, 2, 512] psum pair-tiles (4 banks), PE tile rows
                0/32/64/96 -> all four K=32 matmuls stream concurrently."""
                dst_pool = expa0p if sl == 0 else expa1p
                dst, nkey = (st["a0"], "na0") if sl == 0 else (st["a1"], "na1")
                pss = []
                for t in range(2):
                    ps = eng_ps.tile([128, 2, 512], F32, tag="engps", name="engps")
                    pss.append(ps)
                for t in range(2):
                    for g in range(2):
                        mc = 4 * qd + 2 * t + g
                        ms, pm = CHUNKS[mc]
                        row = 64 * t + 32 * g
                        nc.tensor.matmul(
                            pss[t][:pm, g, :512],
                            k4[row:row + C8, ms:ms + pm],
                            q4[row:row + C8, sl:sl + 512],
                            start=True,
                            stop=True,
                            tile_position=(row, 0),
                        )
                for t in range(2):
                    ept = dst_pool.tile([128, 2, 512], BF16, tag="ep", name="ep")
                    nc.scalar.activation(
                        ept[:, :, :],
                        pss[t][:, :, :],
                        mybir.ActivationFunctionType.Exp,
                    )
                    dst[2 * qd + t] = ept
                    st[nkey] += 1
                    yield ("eng" if t == 0 else None)

            def emit_quad_B(qd, sl, tiles):
                """Same 4-way packing for cols sl:sl+288 of the B half."""
                pss = []
                for t in range(2):
                    ps = eng_ps.tile([128, 2, 512], F32, tag="engps", name="engps")
                    pss.append(ps)
                for t in range(2):
                    for g in range(2):
                        mc = 4 * qd + 2 * t + g
                        ms, pm = CHUNKS[mc]
                        row = 64 * t + 32 * g
                        nc.tensor.matmul(
                            pss[t][:pm, g, :288],
                            k4[row:row + C8, ms:ms + pm],
                            q4[row:row + C8, sl:sl + 288],
                            start=True,
                            stop=True,
                            tile_position=(row, 0),
                        )
                for t in range(2):
                    nc.scalar.activation(
                        tiles[t][:, :, sl - 1024:sl - 1024 + 288],
                        pss[t][:, :, :288],
                        mybir.ActivationFunctionType.Exp,
                    )
                    yield ("eng" if t == 0 else None)

            def emit_lone_A():
                # lone m-chunk (64 rows): both 512-col slices 2-way packed
                ms, pm = CHUNKS[NCH - 1]
                es = eng_ps.tile([128, 2, 512], F32, tag="engps", name="engps")
                for g, sl in enumerate((0, 512)):
                    nc.tensor.matmul(
                        es[:pm, g, :512],
                        k4[32 * g:32 * g + C8, ms:ms + pm],
                        q4[32 * g:32 * g + C8, sl:sl + 512],
                        start=True,
                        stop=True,
                        tile_position=(32 * g, 0),
                    )
                e0 = expa0p.tile([128, 2, 512], BF16, tag="ep", name="ep")
                nc.scalar.activation(
                    e0[:pm, 0, :], es[:pm, 0, :], mybir.ActivationFunctionType.Exp
                )
                nc.gpsimd.memset(e0[pm:128, 0, :], 0.0)
                st["a0"][NPAIR] = e0
                st["na0"] += 1
                yield "eng"
                e1 = expa1p.tile([128, 2, 512], BF16, tag="ep", name="ep")
                nc.scalar.activation(
                    e1[:pm, 0, :], es[:pm, 1, :], mybir.ActivationFunctionType.Exp
                )
                nc.gpsimd.memset(e1[pm:128, 0, :], 0.0)
                st["a1"][NPAIR] = e1
                st["na1"] += 1
                yield

            def emit_lone_B():
                ms, pm = CHUNKS[NCH - 1]
                ept = expbp.tile([128, 2, 576], BF16, tag="epB", name="epB")
                es2 = eng_ps.tile([128, 2, 512], F32, tag="engps", name="engps")
                for h in range(2):
                    nc.tensor.matmul(
                        es2[:pm, h, :288],
                        k4[32 * h:32 * h + C8, ms:ms + pm],
                        q4[32 * h:32 * h + C8, 1024 + 288 * h:1024 + 288 * (h + 1)],
                        start=True, stop=True, tile_position=(32 * h, 0),
                    )
                nc.scalar.activation(
                    ept[:pm, 0, 0:576],
                    es2[:pm, :, :288],
                    mybir.ActivationFunctionType.Exp,
                )
                nc.gpsimd.memset(ept[pm:128, 0, 0:576], 0.0)
                st["bt"][NPAIR] = ept
                st["nbt"] += 1
                yield "eng"

            # --- emission schedule: get a0 + vt complete as early as
            # possible (unlocks the consumer's cross chunks 0-3), then a1,
            # then the B halves.
            q_piece(0, 512); yield
            if u == 0:
                # DMA gate: this copy waits for q4's first slice (i.e. s0
                # landed); only then does the gpsimd ring issue the bulk
                # s2/batch-1 loads, keeping the DMA engines exclusive to the
                # critical s0/s1 transfers during the ramp.
                gate = smallp.tile([1, 2], F16, tag="gate", name="gate")
                nc.gpsimd.tensor_copy(gate[:], q4[0:1, 0:2])
                load_x(0, 2, nc.gpsimd)
                # (A second warmup burst emitted here, behind the stalled q
                # matmuls, measured WORSE (234.7us vs ~226): stalled mms
                # block the in-order PE head rather than moving aside, so
                # the burst serialized after the DMA wait.  The HAM re-
                # throttle during the s0 wait gap is thus unavoidable.)
            k_piece(0, 512); yield
            yield from emit_quad_A(0, 0)
            k_piece(512, 512); yield
            emit_vt(0); yield
            emit_vt(1); yield
            yield from emit_quad_A(1, 0)
            k_piece(1024, 288); yield
            k_piece(1312, 288); yield
            emit_vt(2); yield
            emit_vt(3); yield
            yield from emit_quad_A(2, 0)
            q_piece(512, 512); yield
            emit_vt(4); yield
            emit_vt(5); yield
            yield from emit_lone_A()       # a0 complete (7); a1 has 1
            # vts interleaved with the a1 quads (measured best: 229.8us vs
            # 233.9us for the all-vts-then-all-quads order)
            emit_vt(6); yield
            yield from emit_quad_A(0, 512)
            emit_vt(7); yield
            emit_vt(8); yield
            yield from emit_quad_A(1, 512)
            emit_vt(9); yield
            emit_vt(10); yield
            yield from emit_quad_A(2, 512)
            emit_vt(11); yield
            emit_vt(12); yield
            # --- vt + a0 + a1 complete: consumer chunks 0-7 may run now ---
            q_piece(1024, 288); yield
            q_piece(1312, 288); yield
            btiles = [expbp.tile([128, 2, 576], BF16, tag="epB", name="epB")
                      for _ in range(6)]
            for qd in range(3):
                yield from emit_quad_B(qd, 1024, btiles[2 * qd:2 * qd + 2])
            for qd in range(3):
                yield from emit_quad_B(qd, 1312, btiles[2 * qd:2 * qd + 2])
            for p in range(6):
                st["bt"][p] = btiles[p]
            st["nbt"] += 6
            yield from emit_lone_B()
            # --- bt complete: chunks 8-12 ---
            st["done"] = True

        def a0_ready(u):
            return (u in stage_out and stage_out[u]["na0"] >= NPAIR + 1
                    and len(stage_out[u]["vt"]) >= NCH)

        # --- prologue: run unit 0 until its first cross chunks unlock ---
        g0 = unit_stage_gen(0)
        while not a0_ready(0):
            next(g0)

        feeder = g0
        out_queue = []   # deferred (b, acc, ncidx) output emissions

        def emit_output(b, acc, ncidx):
            ns, pn = CHUNKS[ncidx]
            nc.sync.dma_start(y[b, ns:ns + pn, :], acc[ncidx][:pn, :])

        for u in range(NU):
            b, i = units[u]
            while not a0_ready(u):
                next(feeder)
            st = stage_out[u]
            nxt = unit_stage_gen(u + 1) if u + 1 < NU else None
            feeder = _chain_gens(feeder, nxt)
            if i == 0:
                batch_acc[b] = [None] * NCH
            acc = batch_acc[b]

            def fill_work(k):
                # Pull deferred outputs and next-stage pieces.  At most ONE
                # energy-matmul piece per fill point: its psum WAR (the exp
                # of the quad one back) is then already satisfied, so the 4
                # packed matmuls dispatch dep-free instead of splitting the
                # cross stream mid-chunk (each split costs ~107ns re-entry).
                # Also never prefetch past the next unit's stage.
                nonlocal feeder
                eng_done = False
                for _ in range(k):
                    if out_queue:
                        bb, aa, nn = out_queue.pop(0)
                        emit_output(bb, aa, nn)
                    elif (feeder is not None and not eng_done
                          and not a0_ready(u + 1)):
                        try:
                            v = next(feeder)
                            if v == "eng":
                                eng_done = True
                        except StopIteration:
                            feeder = None
                    else:
                        return

            for ncidx, (ns, pn) in enumerate(CHUNKS):
                if ns >= 1024:
                    while st["nbt"] < NPAIR + 1:
                        next(feeder)
                elif ns >= 512:
                    while st["na1"] < NPAIR + 1:
                        next(feeder)
                cps = cps_ps.tile([128, 512], F32, tag="cps", name="cps")
                for mc in range(NCH):
                    # lone m-chunk is zero-padded to 128 rows: K=64 matmuls
                    # stream at half rate, K=128 at full.
                    p, g = divmod(mc, 2)
                    if mc == NCH - 1:
                        p, g = NPAIR, 0
                    if ns < 512:
                        lhsT = st["a0"][p][:128, g, ns:ns + pn]
                    elif ns < 1024:
                        lhsT = st["a1"][p][:128, g, ns - 512:ns - 512 + pn]
                    else:
                        lhsT = st["bt"][p][:128, g, ns - 1024:ns - 1024 + pn]
                    nc.tensor.matmul(
                        cps[:pn, :C + 1],
                        lhsT,
                        st["vt"][mc][:128, :],
                        start=(mc == 0),
                        stop=(mc == NCH - 1),
                    )
                rinv = smallp.tile([128, 1], F32, tag="rinv", name="rinv")
                nc.vector.reciprocal(rinv[:pn], cps[:pn, C:C + 1])
                if i == 0:
                    acc[ncidx] = accp.tile([128, C], BF16, tag="acc", name="acc")
                    nc.vector.tensor_scalar_mul(
                        acc[ncidx][:pn], cps[:pn, :C], rinv[:pn]
                    )
                else:
                    nc.vector.scalar_tensor_tensor(
                        acc[ncidx][:pn],
                        cps[:pn, :C],
                        rinv[:pn],
                        acc[ncidx][:pn],
                        op0=mybir.AluOpType.mult,
                        op1=mybir.AluOpType.add,
                    )
                if i == 2:
                    if u == NU - 1:
                        # last unit: nothing left to overlap, store now
                        emit_output(b, acc, ncidx)
                    else:
                        out_queue.append((b, acc, ncidx))
                fill_work(6)

        if feeder is not None:
            for _ in feeder:
                pass
        while out_queue:
            bb, aa, nn = out_queue.pop(0)
            emit_output(bb, aa, nn)

    legalize_waits(nc)
    return nc


def _host_prep(Wq, bq, Wk, bk, Wv, bv, fusion_weights):
    """Build the combined, pre-sharded weight tensors.

    wqk  [128, 2, 256]: [p, cc, 0:128]  = tile(Wq.T,(1,4))[p+128cc]
                        [p, cc, 128:256]= tile(Wk.T,(1,4))[p+128cc]
    bqk  [128, 2]     : col0 = tile(bq,4), col1 = tile(bk,4)
    wvta [128, 2, 774]: [p, cc, 258i:+258] = (Wv.T*fw_i/3 | pad2)[p+128cc]
    bvba [128, 774]   : [*, 258i:+258] = (bv*fw_i/3 | 1 | 0) broadcast
    """
    f32, f16 = np.float32, np.float16
    wq4t = np.tile(Wq.T, (1, 4)).astype(f16)       # [256,128]
    wk4t = np.tile(Wk.T, (1, 4)).astype(f16)
    wqk = np.stack([np.concatenate([wq4t[cc * 128:(cc + 1) * 128],
                                    wk4t[cc * 128:(cc + 1) * 128]], axis=1)
                    for cc in range(2)], axis=1)    # [128, 2, 256]
    wqk = np.ascontiguousarray(wqk.reshape(128, 512), dtype=f16)
    bqk = np.ascontiguousarray(
        np.stack([np.tile(bq, 4), np.tile(bk, 4)], axis=1), dtype=f32)  # [128,2]
    wvt_full = np.zeros((C, 3 * (C + 2)), dtype=f16)
    bv_full = np.zeros((1, 3 * (C + 2)), dtype=f32)
    for i in range(3):
        sc = f32(fusion_weights[i]) / f32(3.0)
        wvt_full[:, 258 * i:258 * i + C] = (Wv.T * sc).astype(f16)
        bv_full[0, 258 * i:258 * i + C] = np.asarray(bv, f32) * sc
        bv_full[0, 258 * i + C] = 1.0
    wvta = np.ascontiguousarray(
        np.stack([wvt_full[:128], wvt_full[128:]], axis=1).reshape(128, 2 * 774),
        dtype=f16)
    bvba = np.ascontiguousarray(np.tile(bv_full.astype(f16), (128, 1)))
    return wqk, bqk, wvta, bvba


_PROGRAM_CACHE = {}


def _ensure_ntff_hook():
    """Register the axon NTFF profile hook that the container's antenv lacks."""
    import types

    try:
        from antenv.axon_hooks import get_axon_ntff_profile_hook  # noqa: F401
        return
    except ImportError:
        pass
    if "/root/.axon_site" not in sys.path:
        sys.path.insert(0, "/root/.axon_site")
    from trn_agent_boot.trn_boot import _ntff_profile_via_ctypes

    hook = _ntff_profile_via_ctypes("/opt/axon/libaxon_pjrt.so")
    mod = types.ModuleType("antenv.axon_hooks")
    mod._hook = hook
    mod.get_axon_ntff_profile_hook = lambda: mod._hook
    mod.set_axon_ntff_profile_hook = lambda h: setattr(mod, "_hook", h)
    import antenv

    antenv.axon_hooks = mod
    sys.modules["antenv.axon_hooks"] = mod


def kernel(s0, s1, s2, Wq, bq, Wk, bk, Wv, bv, fusion_weights, _trace=False):
    s0 = np.ascontiguousarray(s0, dtype=np.float16)
    s1 = np.ascontiguousarray(s1, dtype=np.float16)
    s2 = np.ascontiguousarray(s2, dtype=np.float16)
    wqk, bqk, wvta, bvba = _host_prep(
        np.asarray(Wq, np.float32), np.asarray(bq, np.float32),
        np.asarray(Wk, np.float32), np.asarray(bk, np.float32),
        np.asarray(Wv, np.float32), np.asarray(bv, np.float32),
        np.asarray(fusion_weights, np.float32),
    )

    if "nc" not in _PROGRAM_CACHE:
        _PROGRAM_CACHE["nc"] = build_program()
    nc = _PROGRAM_CACHE["nc"]

    # [B, C, N] -> [B, 128, 2*N]: partition p holds channel rows p and p+128
    # side by side, so each (batch, stream) loads with a single dma_start.
    streams = [
        np.ascontiguousarray(
            s.reshape(B, 2, 128, N).transpose(0, 2, 1, 3).reshape(B, 128, 2 * N)
        )
        for s in (s0, s1, s2)
    ]
    in_maps = []
    for core in range(NCORES):
        lo, hi = core * BPC, (core + 1) * BPC
        m = {
            "s0": streams[0][lo:hi],
            "s1": streams[1][lo:hi],
            "s2": streams[2][lo:hi],
            "wqk": wqk, "bqk": bqk, "wvta": wvta, "bvba": bvba,
        }
        in_maps.append(m)

    if _trace:
        _ensure_ntff_hook()
    res = run_bass_kernel_spmd(nc, in_maps, list(range(NCORES)), trace=_trace)
    out = np.concatenate(
        [np.asarray(res.results[c]["y"]).astype(np.float32) for c in range(NCORES)],
        axis=0,
    )
    out = out.transpose(0, 2, 1).reshape(B, C, T, J)
    if _trace:
        kernel.last_exec_time_ns = res.exec_time_ns
        kernel.last_results = res
    return out

